# revision 3
# baseline (speedup 1.0000x reference)
"""Causal single-head attention (B=4, T=2048, D=1024) for 8 TRN2 NeuronCores.

v4: bf16 PE path.  BOTH weight matrices are re-associated off the shared
sequence: A(X Wv) -> (A X) Wv kills the duplicated V-projection, and
Q K^T = Q (X Wk)^T = (Q Wk^T) X^T kills the duplicated K-projection --
G = Q Wk^T covers only this core's 1024 queries, and scores contract
G^T against the raw input X^T.  No cross-core communication anywhere.  AX is accumulated TRANSPOSED
((AX)^T[d,q], lhsT = X key-block, rhs = aT) so the per-tile tail is one
PSUM->SBUF copy + 16 N=512 matmuls, scaled by the Act-accumulated softmax
denominators directly in [q,1] orientation.  256-wide causal key chunks,
balanced q-tile split (even core: tiles 0,2,..,14; odd: 1,3,..,15; both see
n_j = j+1 chunks).  Q resident in SBUF; K-proj chunks emitted 2 ahead of the
attention tile that consumes them; tile tails software-pipelined one behind.

Per-core PE floor: Qproj 65,536 + G=QWk^T 65,536 + G-transp 8,192 +
scores 73,728 + transp 9,216 + AX 73,728 + AX-transp 8,192 +
AXWv 65,536 = 361,472 cyc ~ 150.6us @2.4GHz.
"""

import sys

for _p in ("/opt/trn_rl_repo", "/root/.axon_site/_ro/trn_rl_repo"):
    if _p not in sys.path:
        sys.path.insert(0, _p)

import numpy as np

import concourse.bass as bass
import concourse.tile as tile
import concourse.mybir as mybir
from concourse import bacc
from concourse.masks import make_identity

F32 = mybir.dt.float32
BF16 = mybir.dt.bfloat16

B, T, D = 4, 2048, 1024
DC = D // 128             # 8 contraction chunks of 128
QT_PER_CORE = 8           # q-tiles (128 rows) per core
NEG_INF = -1.0e30


def _emit_body(nc, tc):
    xT_d = nc.xT_d          # [D, T]   x^T (keys layout) bf16
    xR_d = nc.xR_d          # [T, D]   x rows             bf16
    xqT_d = nc.xqT_d        # [D, 1024] my queries' x^T   bf16
    wq_d, wk_d, wv_d = nc.wq_d, nc.wk_d, nc.wv_d
    mask_d, out_d = nc.mask_d, nc.out_d

    with (
        tc.tile_pool(name="pcom", bufs=4, space="PSUM") as pcom,
        tc.tile_pool(name="pax", bufs=1, space="PSUM") as paxp,
        tc.tile_pool(name="pgpo", bufs=1, space="PSUM") as pgpo,
        tc.tile_pool(name="tiny", bufs=1) as tiny,
        tc.tile_pool(name="kt", bufs=1) as ktp,
        tc.tile_pool(name="xr", bufs=1) as xrp,
        tc.tile_pool(name="qt", bufs=1) as qtp,
        tc.tile_pool(name="wv", bufs=1) as wvp,
        tc.tile_pool(name="msk", bufs=1) as mskp,
        tc.tile_pool(name="wrq", bufs=1) as wrq,
        tc.tile_pool(name="xrq", bufs=1) as xrq,
        tc.tile_pool(name="wrk", bufs=1) as wrk,
        tc.tile_pool(name="gt", bufs=1) as gtp,
        tc.tile_pool(name="attn", bufs=5) as attnp,
        tc.tile_pool(name="axs", bufs=2) as axsp,
        tc.tile_pool(name="axts", bufs=2) as axtsp,
        tc.tile_pool(name="outp", bufs=2) as outp,
        tc.tile_pool(name="stats", bufs=16) as stats,
    ):
        # ---- DMAs in consumption-priority order ----
        xT_sb = ktp.tile([128, DC, T], BF16)        # X^T [d_in, key]

        def dma_xt(c):
            # 256-key chunk c of X^T into the resident tile
            nc.sync.dma_start(
                xT_sb[:, :, c * 256:(c + 1) * 256],
                xT_d[:, c * 256:(c + 1) * 256].rearrange(
                    "(c p) t -> p c t", p=128))

        wq_r = wrq.tile([128, DC, D], BF16, tag="w")
        xq_r = xrq.tile([128, DC, 1024], BF16, tag="x")

        def dma_xq(h):
            nc.sync.dma_start(
                xq_r[:, :, h * 512:(h + 1) * 512],
                xqT_d[:, h * 512:(h + 1) * 512].rearrange(
                    "(c p) q -> p c q", p=128))

        dma_xq(0)
        for dc in range(DC):
            nc.sync.dma_start(wq_r[:, dc, :], wq_d[dc * 128:(dc + 1) * 128, :])

        wkT_r = wrk.tile([128, DC, D], BF16, tag="w")
        for dc in range(DC):
            nc.sync.dma_start(wkT_r[:, dc, :], wk_d[dc * 128:(dc + 1) * 128, :])

        msk_sb = mskp.tile([128, QT_PER_CORE, 256], BF16)
        nc.sync.dma_start(msk_sb[:], mask_d.rearrange("j p k -> p j k"))
        dma_xt(0)
        dma_xt(1)

        xR_sb = xrp.tile([128, T // 128, D], BF16)  # X    [key, d]
        nc.sync.dma_start(
            xR_sb[:, 0:4, :],
            xR_d[0:512, :].rearrange("(t p) d -> p t d", p=128))

        dma_xq(1)
        for c in range(2, 8):
            dma_xt(c)
        wv_sb = wvp.tile([128, DC, D], BF16)        # Wv   [d_in, d_out]
        for dc in range(DC):
            nc.sync.dma_start(wv_sb[:, dc, :], wv_d[dc * 128:(dc + 1) * 128, :])

        for s in range(1, 4):
            nc.sync.dma_start(
                xR_sb[:, s * 4:(s + 1) * 4, :],
                xR_d[s * 512:(s + 1) * 512, :].rearrange(
                    "(t p) d -> p t d", p=128))

        ident_f = tiny.tile([128, 128], F32)
        make_identity(nc, ident_f[:])
        ident = tiny.tile([128, 128], BF16)
        nc.vector.tensor_copy(ident[:], ident_f[:])

        qT_sb = qtp.tile([128, DC, 1024], BF16)     # Q^T  [e, q]
        gT_sb = gtp.tile([128, DC, QT_PER_CORE, 128], BF16)  # (Q Wk^T)^T


        def emit_q_proj(h, half):
            pss = [pcom.tile([128, 512], F32, tag="pc", name=f"qps{mm}")
                   for mm in range(4)]
            for dc in range(DC):
                for mm in range(4):
                    m = half * 4 + mm
                    nc.tensor.matmul(
                        pss[mm][:],
                        wq_r[:, dc, m * 128:(m + 1) * 128],
                        xq_r[:, dc, h * 512:(h + 1) * 512],
                        start=(dc == 0), stop=(dc == DC - 1),
                    )
            for mm in range(4):
                nc.vector.tensor_copy(
                    qT_sb[:, half * 4 + mm, h * 512:(h + 1) * 512],
                    pss[mm][:])

        def emit_g_mm(j):
            # G = Q Wk^T for tile j; PSUM -> gs (bf16) halves on DVE
            pg = pgpo.tile([128, 1024], F32, tag="gp", name="pg")
            for ec in range(DC):
                for u in range(2):
                    nc.tensor.matmul(
                        pg[:, u * 512:(u + 1) * 512],
                        qT_sb[:, ec, j * 128:(j + 1) * 128],
                        wkT_r[:, ec, u * 512:(u + 1) * 512],
                        start=(ec == 0), stop=(ec == DC - 1),
                    )
            gs = axsp.tile([128, DC, 128], BF16, tag="axs", name="gs")
            pgv = pg[:].rearrange("p (m d) -> p m d", m=DC)
            for g in range(2):
                nc.vector.tensor_copy(
                    gs[:, g * 4:(g + 1) * 4, :], pgv[:, g * 4:(g + 1) * 4, :])
            return gs

        def emit_g_tr(j, gs, g):
            # transpose one gs half into gT_sb[:, :, j, :]
            ps_t = pcom.tile([128, 512], BF16, tag="pc", name="ps_tg")
            for mm in range(4):
                nc.tensor.matmul(
                    ps_t[:, mm * 128:(mm + 1) * 128],
                    gs[:, g * 4 + mm, :],
                    ident[:],
                    is_transpose=True,
                    start=True, stop=True,
                )
            nc.scalar.copy(
                gT_sb[:, g * 4:(g + 1) * 4, j, :],
                ps_t[:].rearrange("p (m q) -> p m q", m=4))

        def emit_g(j):
            gs = emit_g_mm(j)
            emit_g_tr(j, gs, 0)
            emit_g_tr(j, gs, 1)

        def emit_q_proj_mouter(h):
            for m in range(DC):
                ps = pcom.tile([128, 512], F32, tag="pc", name="qpm")
                for dc in range(DC):
                    nc.tensor.matmul(
                        ps[:],
                        wq_r[:, dc, m * 128:(m + 1) * 128],
                        xq_r[:, dc, h * 512:(h + 1) * 512],
                        start=(dc == 0), stop=(dc == DC - 1),
                    )
                nc.vector.tensor_copy(
                    qT_sb[:, m, h * 512:(h + 1) * 512], ps[:])

        emit_q_proj(0, 0)
        emit_q_proj(0, 1)
        emit_g(0)
        emit_g(1)

        def emit_scores(j, c):
            # scores chunk c -> exp (+row sum)
            nj = j + 1
            ps = pcom.tile([128, 512], F32, tag="pc", name="st")[:, 0:256]
            for dc in range(DC):
                nc.tensor.matmul(
                    ps[:],
                    gT_sb[:, dc, j, :],
                    xT_sb[:, dc, c * 256:(c + 1) * 256],
                    start=(dc == 0), stop=(dc == DC - 1),
                )
            if c == nj - 1:
                nc.vector.tensor_add(ps[:], ps[:], msk_sb[:, j, :])
            at = attnp.tile([128, 256], BF16, tag="attn")
            s_c = stats.tile([128, 1], F32, tag="mx")
            nc.scalar.activation(
                out=at[:], in_=ps[:],
                func=mybir.ActivationFunctionType.Exp,
                bias=0.0, scale=1.0,
                accum_out=s_c[:],
            )
            return at, s_c

        def emit_post(j, c, at, pax):
            # transpose exp chunk -> aT, accumulate A@X [q, d]
            nj = j + 1
            ps_t = pcom.tile([128, 512], BF16, tag="pc", name="ps_t")[:, 0:256]
            for b in range(2):
                nc.tensor.matmul(
                    ps_t[:, b * 128:(b + 1) * 128],
                    at[:, b * 128:(b + 1) * 128],
                    ident[:],
                    is_transpose=True,
                    start=True, stop=True,
                )
            aT = attnp.tile([128, 2, 128], BF16, tag="attn", name="aT")
            nc.vector.tensor_copy(
                aT[:], ps_t[:].rearrange("p (b q) -> p b q", b=2))
            for bb in range(2):
                kb = c * 2 + bb
                for u in range(2):
                    nc.tensor.matmul(
                        pax[:, u * 512:(u + 1) * 512],
                        aT[:, bb, :],
                        xR_sb[:, kb, u * 512:(u + 1) * 512],
                        start=(kb == 0), stop=(kb == 2 * nj - 1),
                    )

        def emit_tail(j, axs, sums):
            # combine row sums -> 1/denom  [q,1]
            while len(sums) > 1:
                nxt = []
                for i in range(0, len(sums) - 1, 2):
                    s_c = stats.tile([128, 1], F32, tag="mx")
                    nc.vector.tensor_add(s_c[:], sums[i][:], sums[i + 1][:])
                    nxt.append(s_c)
                if len(sums) % 2:
                    nxt.append(sums[-1])
                sums = nxt
            recip = stats.tile([128, 1], F32, tag="mx")
            nc.vector.reciprocal(recip[:], sums[0][:])

            # transpose A@X -> (AX)^T [d, q] and (A X) Wv, pipelined per
            # dc-half so the final tail's copies overlap PE work
            axt = axtsp.tile([128, DC, 128], BF16, tag="axt")
            po = pgpo.tile([128, DC, 128], F32, tag="gp", name="po")
            pof = po[:].rearrange("p m q -> p (m q)")
            for g in range(2):
                ps_t = pcom.tile([128, 512], BF16, tag="pc", name="ps_tx")
                for mm in range(4):
                    nc.tensor.matmul(
                        ps_t[:, mm * 128:(mm + 1) * 128],
                        axs[:, g * 4 + mm, :],
                        ident[:],
                        is_transpose=True,
                        start=True, stop=True,
                    )
                nc.vector.tensor_copy(
                    axt[:, g * 4:(g + 1) * 4, :],
                    ps_t[:].rearrange("p (m q) -> p m q", m=4))
                for dc in range(g * 4, g * 4 + 4):
                    for u in range(2):
                        nc.tensor.matmul(
                            pof[:, u * 512:(u + 1) * 512],
                            axt[:, dc, :],
                            wv_sb[:, dc, u * 512:(u + 1) * 512],
                            start=(dc == 0), stop=(dc == DC - 1),
                        )
            ot = outp.tile([128, D], BF16, tag="o")
            for u in range(2):
                nc.vector.tensor_scalar_mul(
                    ot[:, u * 512:(u + 1) * 512],
                    pof[:, u * 512:(u + 1) * 512], recip[:])
            nc.sync.dma_start(out_d[j * 128:(j + 1) * 128, :], ot[:])

        pending = None
        for step in range(QT_PER_CORE):
            if step == 2:
                emit_q_proj_mouter(1)
            fillers = []
            if step + 2 < QT_PER_CORE:
                jn = step + 2
                gs_n = emit_g_mm(jn)
                fillers = [(jn, gs_n, 0), (jn, gs_n, 1)]
            # staggered chunk pipeline: scores(c+1) + G transposes cover
            # exp(c) latency
            pax = paxp.tile([128, 1024], F32, tag="ax", name="pax")
            sums = []
            prev = None
            for c in range(step + 1):
                at, s_c = emit_scores(step, c)
                sums.append(s_c)
                if fillers:
                    emit_g_tr(*fillers.pop(0))
                if prev is not None:
                    emit_post(step, prev[0], prev[1], pax)
                prev = (c, at)
            for f in fillers:
                emit_g_tr(*f)
            if pending is not None:
                emit_tail(*pending)          # covers the last chunk's exp
            emit_post(step, prev[0], prev[1], pax)
            axs = axsp.tile([128, DC, 128], BF16, tag="axs")
            paxv = pax[:].rearrange("p (m d) -> p m d", m=DC)
            for g in range(2):
                nc.scalar.copy(
                    axs[:, g * 4:(g + 1) * 4, :], paxv[:, g * 4:(g + 1) * 4, :])
            pending = (step, axs, sums)
        emit_tail(*pending)


def build_nc(reps=1):
    nc = bacc.Bacc("TRN2", target_bir_lowering=False, debug=False,
                   num_swdge_queues=4)

    nc.xT_d = nc.dram_tensor("xT", [D, T], BF16, kind="ExternalInput")
    nc.xR_d = nc.dram_tensor("xR", [T, D], BF16, kind="ExternalInput")
    nc.xqT_d = nc.dram_tensor("xqT", [D, 1024], BF16, kind="ExternalInput")
    nc.wq_d = nc.dram_tensor("wq", [D, D], BF16, kind="ExternalInput")
    nc.wk_d = nc.dram_tensor("wk", [D, D], BF16, kind="ExternalInput")
    nc.wv_d = nc.dram_tensor("wv", [D, D], BF16, kind="ExternalInput")
    nc.mask_d = nc.dram_tensor("mask", [QT_PER_CORE, 128, 256], BF16,
                               kind="ExternalInput")
    nc.out_d = nc.dram_tensor("out", [1024, D], BF16, kind="ExternalOutput")

    with tile.TileContext(nc) as tc:
        for _rep in range(reps):
            _emit_body(nc, tc)

    nc.compile()
    return nc


def make_in_maps(input_vector, w_q, w_k, w_v):
    import ml_dtypes
    bf16 = ml_dtypes.bfloat16

    input_vector = np.asarray(input_vector, dtype=np.float32)
    wq = np.ascontiguousarray(
        (np.asarray(w_q, dtype=np.float32) / np.sqrt(np.float32(D))
         ).astype(bf16))
    wkT = np.ascontiguousarray(np.asarray(w_k, dtype=np.float32).T.astype(bf16))
    wv = np.ascontiguousarray(np.asarray(w_v, dtype=np.float32).astype(bf16))

    in_maps = []
    qrows_per_core = []
    for core in range(8):
        b = core // 2
        par = core % 2
        qt = [2 * j + par for j in range(QT_PER_CORE)]
        xb = input_vector[b]                                   # [T, D]
        xT = np.ascontiguousarray(xb.T.astype(bf16))           # [D, T]
        xR = np.ascontiguousarray(xb.astype(bf16))             # [T, D]
        qrows = np.concatenate(
            [np.arange(t * 128, (t + 1) * 128) for t in qt])
        xqT = np.ascontiguousarray(xb[qrows].T.astype(bf16))   # [D, 1024]
        mask = np.zeros((QT_PER_CORE, 128, 256), np.float32)
        for j, t in enumerate(qt):
            k0 = j * 256                  # last chunk covers keys [k0, k0+256)
            q_abs = t * 128 + np.arange(128)[:, None]
            k_abs = k0 + np.arange(256)[None, :]
            mask[j] = np.where(k_abs <= q_abs, 0.0, np.float32(NEG_INF))
        in_maps.append({
            "xT": xT, "xR": xR, "xqT": xqT,
            "wq": wq, "wk": wkT, "wv": wv, "mask": mask.astype(bf16),
        })
        qrows_per_core.append((b, qrows))
    return in_maps, qrows_per_core


def assemble_output(results, qrows_per_core):
    out = np.empty((B, T, D), np.float32)
    for core, (b, qrows) in enumerate(qrows_per_core):
        out[b, qrows] = results[core]["out"].astype(np.float32)
    return out


_NC_CACHE = {}


def kernel(input_vector, w_q, w_k, w_v):
    """Full-input entry point: shards across 8 NeuronCores, returns the
    full [4, 2048, 1024] float32 attention output."""
    from concourse.bass_utils import run_bass_kernel_spmd

    if "nc" not in _NC_CACHE:
        _NC_CACHE["nc"] = build_nc()
    nc = _NC_CACHE["nc"]
    in_maps, qrc = make_in_maps(input_vector, w_q, w_k, w_v)
    res = run_bass_kernel_spmd(nc, in_maps, core_ids=list(range(8)))
    return assemble_output(res.results, qrc)


# revision 4
# speedup vs baseline: 1.1840x; 1.1840x over previous
"""Causal single-head attention (B=4, T=2048, D=1024) for 8 TRN2 NeuronCores.

v5: bf16 PE path.  W2 = (Wq/sqrt(D)) Wk^T is fused on the host (static
weight transform), so G = Xq W2 directly and the Q-projection phase
disappears.  BOTH weight matrices are re-associated off the shared
sequence: A(X Wv) -> (A X) Wv kills the duplicated V-projection, and
Q K^T = Q (X Wk)^T = (Q Wk^T) X^T kills the duplicated K-projection --
G = Q Wk^T covers only this core's 1024 queries, and scores contract
G^T against the raw input X^T.  No cross-core communication anywhere.  AX is accumulated TRANSPOSED
((AX)^T[d,q], lhsT = X key-block, rhs = aT) so the per-tile tail is one
PSUM->SBUF copy + 16 N=512 matmuls, scaled by the Act-accumulated softmax
denominators directly in [q,1] orientation.  256-wide causal key chunks,
balanced q-tile split (even core: tiles 0,2,..,14; odd: 1,3,..,15; both see
n_j = j+1 chunks).  Q resident in SBUF; K-proj chunks emitted 2 ahead of the
attention tile that consumes them; tile tails software-pipelined one behind.

Per-core PE floor: Qproj 65,536 + G=QWk^T 65,536 + G-transp 8,192 +
scores 73,728 + transp 9,216 + AX 73,728 + AX-transp 8,192 +
AXWv 65,536 = 361,472 cyc ~ 150.6us @2.4GHz.
"""

import sys

for _p in ("/opt/trn_rl_repo", "/root/.axon_site/_ro/trn_rl_repo"):
    if _p not in sys.path:
        sys.path.insert(0, _p)

import numpy as np

import concourse.bass as bass
import concourse.tile as tile
import concourse.mybir as mybir
from concourse import bacc
from concourse.masks import make_identity

F32 = mybir.dt.float32
BF16 = mybir.dt.bfloat16

B, T, D = 4, 2048, 1024
DC = D // 128             # 8 contraction chunks of 128
QT_PER_CORE = 8           # q-tiles (128 rows) per core
NEG_INF = -1.0e30


def _emit_body(nc, tc):
    xT_d = nc.xT_d          # [D, T]   x^T (keys layout) bf16
    xR_d = nc.xR_d          # [T, D]   x rows             bf16
    xqT_d = nc.xqT_d        # [D, 1024] my queries' x^T   bf16
    w2_d, wv_d = nc.w2_d, nc.wv_d
    mask_d, out_d = nc.mask_d, nc.out_d

    with (
        tc.tile_pool(name="pcom", bufs=4, space="PSUM") as pcom,
        tc.tile_pool(name="pax", bufs=1, space="PSUM") as paxp,
        tc.tile_pool(name="pgpo", bufs=1, space="PSUM") as pgpo,
        tc.tile_pool(name="tiny", bufs=1) as tiny,
        tc.tile_pool(name="kt", bufs=1) as ktp,
        tc.tile_pool(name="xr", bufs=1) as xrp,
        tc.tile_pool(name="qt", bufs=1) as qtp,
        tc.tile_pool(name="wv", bufs=1) as wvp,
        tc.tile_pool(name="msk", bufs=1) as mskp,
        tc.tile_pool(name="wrq", bufs=1) as wrq,
        tc.tile_pool(name="xrq", bufs=1) as xrq,
        tc.tile_pool(name="wrk", bufs=1) as wrk,
        tc.tile_pool(name="gt", bufs=1) as gtp,
        tc.tile_pool(name="attn", bufs=5) as attnp,
        tc.tile_pool(name="axs", bufs=2) as axsp,
        tc.tile_pool(name="axts", bufs=2) as axtsp,
        tc.tile_pool(name="outp", bufs=2) as outp,
        tc.tile_pool(name="stats", bufs=16) as stats,
    ):
        # ---- DMAs in consumption-priority order ----
        xT_sb = ktp.tile([128, DC, T], BF16)        # X^T [d_in, key]

        def dma_xt(c):
            # 256-key chunk c of X^T into the resident tile
            nc.sync.dma_start(
                xT_sb[:, :, c * 256:(c + 1) * 256],
                xT_d[:, c * 256:(c + 1) * 256].rearrange(
                    "(c p) t -> p c t", p=128))

        xq_r = xrq.tile([128, DC, 1024], BF16, tag="x")

        def dma_xq(h):
            nc.sync.dma_start(
                xq_r[:, :, h * 512:(h + 1) * 512],
                xqT_d[:, h * 512:(h + 1) * 512].rearrange(
                    "(c p) q -> p c q", p=128))

        dma_xq(0)
        w2_r = wrk.tile([128, DC, D], BF16, tag="w")
        for dc in range(DC):
            nc.sync.dma_start(w2_r[:, dc, :], w2_d[dc * 128:(dc + 1) * 128, :])

        msk_sb = mskp.tile([128, QT_PER_CORE, 256], BF16)
        nc.sync.dma_start(msk_sb[:], mask_d.rearrange("j p k -> p j k"))
        dma_xt(0)
        dma_xt(1)

        xR_sb = xrp.tile([128, T // 128, D], BF16)  # X    [key, d]
        nc.sync.dma_start(
            xR_sb[:, 0:4, :],
            xR_d[0:512, :].rearrange("(t p) d -> p t d", p=128))

        dma_xq(1)
        for c in range(2, 8):
            dma_xt(c)
        wv_sb = wvp.tile([128, DC, D], BF16)        # Wv   [d_in, d_out]
        for dc in range(DC):
            nc.sync.dma_start(wv_sb[:, dc, :], wv_d[dc * 128:(dc + 1) * 128, :])

        for s in range(1, 4):
            nc.sync.dma_start(
                xR_sb[:, s * 4:(s + 1) * 4, :],
                xR_d[s * 512:(s + 1) * 512, :].rearrange(
                    "(t p) d -> p t d", p=128))

        ident_f = tiny.tile([128, 128], F32)
        make_identity(nc, ident_f[:])
        ident = tiny.tile([128, 128], BF16)
        nc.vector.tensor_copy(ident[:], ident_f[:])

        gT_sb = gtp.tile([128, DC, QT_PER_CORE, 128], BF16)  # (Q Wk^T)^T


        def emit_g_mm(j):
            # G = Q Wk^T for tile j; PSUM -> gs (bf16) halves on DVE
            pg = pgpo.tile([128, 1024], F32, tag="gp", name="pg")
            for ec in range(DC):
                for u in range(2):
                    nc.tensor.matmul(
                        pg[:, u * 512:(u + 1) * 512],
                        xq_r[:, ec, j * 128:(j + 1) * 128],
                        w2_r[:, ec, u * 512:(u + 1) * 512],
                        start=(ec == 0), stop=(ec == DC - 1),
                    )
            gs = axsp.tile([128, DC, 128], BF16, tag="axs", name="gs")
            pgv = pg[:].rearrange("p (m d) -> p m d", m=DC)
            for g in range(2):
                nc.vector.tensor_copy(
                    gs[:, g * 4:(g + 1) * 4, :], pgv[:, g * 4:(g + 1) * 4, :])
            return gs

        def emit_g_tr(j, gs, g):
            # transpose one gs half into gT_sb[:, :, j, :]
            ps_t = pcom.tile([128, 512], BF16, tag="pc", name="ps_tg")
            for mm in range(4):
                nc.tensor.matmul(
                    ps_t[:, mm * 128:(mm + 1) * 128],
                    gs[:, g * 4 + mm, :],
                    ident[:],
                    is_transpose=True,
                    start=True, stop=True,
                )
            nc.scalar.copy(
                gT_sb[:, g * 4:(g + 1) * 4, j, :],
                ps_t[:].rearrange("p (m q) -> p m q", m=4))

        def emit_g(j):
            gs = emit_g_mm(j)
            emit_g_tr(j, gs, 0)
            emit_g_tr(j, gs, 1)

        emit_g(0)
        emit_g(1)

        def emit_scores(j, c):
            # scores chunk c -> exp (+row sum)
            nj = j + 1
            ps = pcom.tile([128, 512], F32, tag="pc", name="st")[:, 0:256]
            for dc in range(DC):
                nc.tensor.matmul(
                    ps[:],
                    gT_sb[:, dc, j, :],
                    xT_sb[:, dc, c * 256:(c + 1) * 256],
                    start=(dc == 0), stop=(dc == DC - 1),
                )
            if c == nj - 1:
                nc.vector.tensor_add(ps[:], ps[:], msk_sb[:, j, :])
            at = attnp.tile([128, 256], BF16, tag="attn")
            s_c = stats.tile([128, 1], F32, tag="mx")
            nc.scalar.activation(
                out=at[:], in_=ps[:],
                func=mybir.ActivationFunctionType.Exp,
                bias=0.0, scale=1.0,
                accum_out=s_c[:],
            )
            return at, s_c

        def emit_post(j, c, at, pax):
            # transpose exp chunk -> aT, accumulate A@X [q, d]
            nj = j + 1
            ps_t = pcom.tile([128, 512], BF16, tag="pc", name="ps_t")[:, 0:256]
            for b in range(2):
                nc.tensor.matmul(
                    ps_t[:, b * 128:(b + 1) * 128],
                    at[:, b * 128:(b + 1) * 128],
                    ident[:],
                    is_transpose=True,
                    start=True, stop=True,
                )
            aT = attnp.tile([128, 2, 128], BF16, tag="attn", name="aT")
            nc.vector.tensor_copy(
                aT[:], ps_t[:].rearrange("p (b q) -> p b q", b=2))
            for bb in range(2):
                kb = c * 2 + bb
                for u in range(2):
                    nc.tensor.matmul(
                        pax[:, u * 512:(u + 1) * 512],
                        aT[:, bb, :],
                        xR_sb[:, kb, u * 512:(u + 1) * 512],
                        start=(kb == 0), stop=(kb == 2 * nj - 1),
                    )

        def emit_tail(j, axs, sums):
            # combine row sums -> 1/denom  [q,1]
            while len(sums) > 1:
                nxt = []
                for i in range(0, len(sums) - 1, 2):
                    s_c = stats.tile([128, 1], F32, tag="mx")
                    nc.vector.tensor_add(s_c[:], sums[i][:], sums[i + 1][:])
                    nxt.append(s_c)
                if len(sums) % 2:
                    nxt.append(sums[-1])
                sums = nxt
            recip = stats.tile([128, 1], F32, tag="mx")
            nc.vector.reciprocal(recip[:], sums[0][:])

            # transpose A@X -> (AX)^T [d, q] and (A X) Wv, pipelined per
            # dc-half so the final tail's copies overlap PE work
            axt = axtsp.tile([128, DC, 128], BF16, tag="axt")
            po = pgpo.tile([128, DC, 128], F32, tag="gp", name="po")
            pof = po[:].rearrange("p m q -> p (m q)")
            for g in range(2):
                ps_t = pcom.tile([128, 512], BF16, tag="pc", name="ps_tx")
                for mm in range(4):
                    nc.tensor.matmul(
                        ps_t[:, mm * 128:(mm + 1) * 128],
                        axs[:, g * 4 + mm, :],
                        ident[:],
                        is_transpose=True,
                        start=True, stop=True,
                    )
                nc.vector.tensor_copy(
                    axt[:, g * 4:(g + 1) * 4, :],
                    ps_t[:].rearrange("p (m q) -> p m q", m=4))
                for dc in range(g * 4, g * 4 + 4):
                    for u in range(2):
                        nc.tensor.matmul(
                            pof[:, u * 512:(u + 1) * 512],
                            axt[:, dc, :],
                            wv_sb[:, dc, u * 512:(u + 1) * 512],
                            start=(dc == 0), stop=(dc == DC - 1),
                        )
            ot = outp.tile([128, D], BF16, tag="o")
            for u in range(2):
                nc.vector.tensor_scalar_mul(
                    ot[:, u * 512:(u + 1) * 512],
                    pof[:, u * 512:(u + 1) * 512], recip[:])
            nc.sync.dma_start(out_d[j * 128:(j + 1) * 128, :], ot[:])

        pending = None
        for step in range(QT_PER_CORE):
            fillers = []
            if step + 2 < QT_PER_CORE:
                jn = step + 2
                gs_n = emit_g_mm(jn)
                fillers = [(jn, gs_n, 0), (jn, gs_n, 1)]
            # staggered chunk pipeline: scores(c+1) + G transposes cover
            # exp(c) latency
            pax = paxp.tile([128, 1024], F32, tag="ax", name="pax")
            sums = []
            prev = None
            for c in range(step + 1):
                at, s_c = emit_scores(step, c)
                sums.append(s_c)
                if fillers:
                    emit_g_tr(*fillers.pop(0))
                if prev is not None:
                    emit_post(step, prev[0], prev[1], pax)
                prev = (c, at)
            for f in fillers:
                emit_g_tr(*f)
            if pending is not None:
                emit_tail(*pending)          # covers the last chunk's exp
            emit_post(step, prev[0], prev[1], pax)
            axs = axsp.tile([128, DC, 128], BF16, tag="axs")
            paxv = pax[:].rearrange("p (m d) -> p m d", m=DC)
            for g in range(2):
                nc.scalar.copy(
                    axs[:, g * 4:(g + 1) * 4, :], paxv[:, g * 4:(g + 1) * 4, :])
            pending = (step, axs, sums)
        emit_tail(*pending)


def build_nc(reps=1):
    nc = bacc.Bacc("TRN2", target_bir_lowering=False, debug=False,
                   num_swdge_queues=4)

    nc.xT_d = nc.dram_tensor("xT", [D, T], BF16, kind="ExternalInput")
    nc.xR_d = nc.dram_tensor("xR", [T, D], BF16, kind="ExternalInput")
    nc.xqT_d = nc.dram_tensor("xqT", [D, 1024], BF16, kind="ExternalInput")
    nc.w2_d = nc.dram_tensor("w2", [D, D], BF16, kind="ExternalInput")
    nc.wv_d = nc.dram_tensor("wv", [D, D], BF16, kind="ExternalInput")
    nc.mask_d = nc.dram_tensor("mask", [QT_PER_CORE, 128, 256], BF16,
                               kind="ExternalInput")
    nc.out_d = nc.dram_tensor("out", [1024, D], BF16, kind="ExternalOutput")

    with tile.TileContext(nc) as tc:
        for _rep in range(reps):
            _emit_body(nc, tc)

    nc.compile()
    return nc


def make_in_maps(input_vector, w_q, w_k, w_v):
    import ml_dtypes
    bf16 = ml_dtypes.bfloat16

    input_vector = np.asarray(input_vector, dtype=np.float32)
    w2 = np.ascontiguousarray(
        ((np.asarray(w_q, dtype=np.float32) / np.sqrt(np.float32(D)))
         @ np.asarray(w_k, dtype=np.float32).T).astype(bf16))
    wv = np.ascontiguousarray(np.asarray(w_v, dtype=np.float32).astype(bf16))

    in_maps = []
    qrows_per_core = []
    for core in range(8):
        b = core // 2
        par = core % 2
        qt = [2 * j + par for j in range(QT_PER_CORE)]
        xb = input_vector[b]                                   # [T, D]
        xT = np.ascontiguousarray(xb.T.astype(bf16))           # [D, T]
        xR = np.ascontiguousarray(xb.astype(bf16))             # [T, D]
        qrows = np.concatenate(
            [np.arange(t * 128, (t + 1) * 128) for t in qt])
        xqT = np.ascontiguousarray(xb[qrows].T.astype(bf16))   # [D, 1024]
        mask = np.zeros((QT_PER_CORE, 128, 256), np.float32)
        for j, t in enumerate(qt):
            k0 = j * 256                  # last chunk covers keys [k0, k0+256)
            q_abs = t * 128 + np.arange(128)[:, None]
            k_abs = k0 + np.arange(256)[None, :]
            mask[j] = np.where(k_abs <= q_abs, 0.0, np.float32(NEG_INF))
        in_maps.append({
            "xT": xT, "xR": xR, "xqT": xqT,
            "w2": w2, "wv": wv, "mask": mask.astype(bf16),
        })
        qrows_per_core.append((b, qrows))
    return in_maps, qrows_per_core


def assemble_output(results, qrows_per_core):
    out = np.empty((B, T, D), np.float32)
    for core, (b, qrows) in enumerate(qrows_per_core):
        out[b, qrows] = results[core]["out"].astype(np.float32)
    return out


_NC_CACHE = {}


def kernel(input_vector, w_q, w_k, w_v):
    """Full-input entry point: shards across 8 NeuronCores, returns the
    full [4, 2048, 1024] float32 attention output."""
    from concourse.bass_utils import run_bass_kernel_spmd

    if "nc" not in _NC_CACHE:
        _NC_CACHE["nc"] = build_nc()
    nc = _NC_CACHE["nc"]
    in_maps, qrc = make_in_maps(input_vector, w_q, w_k, w_v)
    res = run_bass_kernel_spmd(nc, in_maps, core_ids=list(range(8)))
    return assemble_output(res.results, qrc)


# revision 5
# speedup vs baseline: 1.5563x; 1.3144x over previous
"""Causal single-head attention (B=4, T=2048, D=1024) for 8 TRN2 NeuronCores.

v5: bf16 PE path.  W2 = (Wq/sqrt(D)) Wk^T is fused on the host (static
weight transform), so G = Xq W2 directly and the Q-projection phase
disappears.  BOTH weight matrices are re-associated off the shared
sequence: A(X Wv) -> (A X) Wv kills the duplicated V-projection, and
Q K^T = Q (X Wk)^T = (Q Wk^T) X^T kills the duplicated K-projection --
G = Q Wk^T covers only this core's 1024 queries, and scores contract
G^T against the raw input X^T.  No cross-core communication anywhere.  AX is accumulated TRANSPOSED
((AX)^T[d,q], lhsT = X key-block, rhs = aT) so the per-tile tail is one
PSUM->SBUF copy + 16 N=512 matmuls, scaled by the Act-accumulated softmax
denominators directly in [q,1] orientation.  256-wide causal key chunks,
balanced q-tile split (even core: tiles 0,2,..,14; odd: 1,3,..,15; both see
n_j = j+1 chunks).  Q resident in SBUF; K-proj chunks emitted 2 ahead of the
attention tile that consumes them; tile tails software-pipelined one behind.

Per-core PE floor: Qproj 65,536 + G=QWk^T 65,536 + G-transp 8,192 +
scores 73,728 + transp 9,216 + AX 73,728 + AX-transp 8,192 +
AXWv 65,536 = 361,472 cyc ~ 150.6us @2.4GHz.
"""

import sys

for _p in ("/opt/trn_rl_repo", "/root/.axon_site/_ro/trn_rl_repo"):
    if _p not in sys.path:
        sys.path.insert(0, _p)

import numpy as np

import concourse.bass as bass
import concourse.tile as tile
import concourse.mybir as mybir
from concourse import bacc
from concourse.masks import make_identity

F32 = mybir.dt.float32
BF16 = mybir.dt.bfloat16

B, T, D = 4, 2048, 1024
DC = D // 128             # 8 contraction chunks of 128
QT_PER_CORE = 8           # q-tiles (128 rows) per core
NEG_INF = -1.0e30


def _emit_body(nc, tc):
    xT_d = nc.xT_d          # [D, T]   x^T (keys layout) bf16
    xR_d = nc.xR_d          # [T, D]   x rows             bf16
    xqT_d = nc.xqT_d        # [D, 1024] my queries' x^T   bf16
    w2_d, wv_d = nc.w2_d, nc.wv_d
    mask_d, out_d = nc.mask_d, nc.out_d

    with (
        tc.tile_pool(name="pcom", bufs=4, space="PSUM") as pcom,
        tc.tile_pool(name="pax", bufs=1, space="PSUM") as paxp,
        tc.tile_pool(name="pgpo", bufs=1, space="PSUM") as pgpo,
        tc.tile_pool(name="tiny", bufs=1) as tiny,
        tc.tile_pool(name="kt", bufs=1) as ktp,
        tc.tile_pool(name="xr", bufs=1) as xrp,
        tc.tile_pool(name="qt", bufs=1) as qtp,
        tc.tile_pool(name="wv", bufs=1) as wvp,
        tc.tile_pool(name="msk", bufs=1) as mskp,
        tc.tile_pool(name="wrq", bufs=1) as wrq,
        tc.tile_pool(name="xrq", bufs=1) as xrq,
        tc.tile_pool(name="wrk", bufs=1) as wrk,
        tc.tile_pool(name="gt", bufs=1) as gtp,
        tc.tile_pool(name="attn", bufs=5) as attnp,
        tc.tile_pool(name="axs", bufs=2) as axsp,
        tc.tile_pool(name="axts", bufs=2) as axtsp,
        tc.tile_pool(name="outp", bufs=2) as outp,
        tc.tile_pool(name="stats", bufs=16) as stats,
    ):
        # ---- DMAs in consumption-priority order ----
        xT_sb = ktp.tile([128, DC, T], BF16)        # X^T [d_in, key]

        def dma_xt(c):
            # 256-key chunk c of X^T into the resident tile
            nc.sync.dma_start(
                xT_sb[:, :, c * 256:(c + 1) * 256],
                xT_d[:, c * 256:(c + 1) * 256].rearrange(
                    "(c p) t -> p c t", p=128))

        xq_r = xrq.tile([128, DC, 1024], BF16, tag="x")

        def dma_xq(h):
            nc.sync.dma_start(
                xq_r[:, :, h * 512:(h + 1) * 512],
                xqT_d[:, h * 512:(h + 1) * 512].rearrange(
                    "(c p) q -> p c q", p=128))

        dma_xq(0)
        w2_r = wrk.tile([128, DC, D], BF16, tag="w")
        for dc in range(DC):
            nc.sync.dma_start(w2_r[:, dc, :], w2_d[dc * 128:(dc + 1) * 128, :])

        msk_sb = mskp.tile([128, QT_PER_CORE, 256], BF16)
        nc.sync.dma_start(msk_sb[:], mask_d.rearrange("j p k -> p j k"))
        dma_xt(0)
        dma_xt(1)

        xR_sb = xrp.tile([128, T // 128, D], BF16)  # X    [key, d]
        nc.sync.dma_start(
            xR_sb[:, 0:4, :],
            xR_d[0:512, :].rearrange("(t p) d -> p t d", p=128))

        dma_xq(1)
        for c in range(2, 8):
            dma_xt(c)
        wv_sb = wvp.tile([128, DC, D], BF16)        # Wv   [d_in, d_out]
        for dc in range(DC):
            nc.sync.dma_start(wv_sb[:, dc, :], wv_d[dc * 128:(dc + 1) * 128, :])

        for s in range(1, 4):
            nc.sync.dma_start(
                xR_sb[:, s * 4:(s + 1) * 4, :],
                xR_d[s * 512:(s + 1) * 512, :].rearrange(
                    "(t p) d -> p t d", p=128))

        ident_f = tiny.tile([128, 128], F32)
        make_identity(nc, ident_f[:])
        ident = tiny.tile([128, 128], BF16)
        nc.vector.tensor_copy(ident[:], ident_f[:])

        gT_sb = gtp.tile([128, DC, QT_PER_CORE, 128], BF16)  # (Q Wk^T)^T


        def emit_g(j):
            # gT = (Xq W2)^T computed directly: out[i,q] via lhsT=W2 chunk,
            # rhs=xq block.  4 concurrent [128,128] groups per pass, each in
            # its own PSUM bank (bank-aligned offsets inside the two big
            # pool tiles); 2 passes cover i-chunks 0-3 / 4-7.
            for p in range(2):
                pga = paxp.tile([128, 1024], F32, tag="ax", name="pga")
                pgb = pgpo.tile([128, DC, 128], F32, tag="gp", name="pgb")
                pgbf = pgb[:].rearrange("p m q -> p (m q)")
                regions = [pga[:, 0:128], pga[:, 512:640],
                           pgbf[:, 0:128], pgbf[:, 512:640]]
                for dc in range(DC):
                    for g in range(4):
                        ic = p * 4 + g
                        nc.tensor.matmul(
                            regions[g],
                            w2_r[:, dc, ic * 128:(ic + 1) * 128],
                            xq_r[:, dc, j * 128:(j + 1) * 128],
                            start=(dc == 0), stop=(dc == DC - 1),
                        )
                for g in range(4):
                    ic = p * 4 + g
                    if g % 2 == 0:
                        nc.scalar.copy(gT_sb[:, ic, j, :], regions[g])
                    else:
                        nc.vector.tensor_copy(gT_sb[:, ic, j, :], regions[g])

        emit_g(0)
        emit_g(1)

        def emit_scores(j, c):
            # scores chunk c -> exp (+row sum)
            nj = j + 1
            ps = pcom.tile([128, 512], F32, tag="pc", name="st")[:, 0:256]
            for dc in range(DC):
                nc.tensor.matmul(
                    ps[:],
                    gT_sb[:, dc, j, :],
                    xT_sb[:, dc, c * 256:(c + 1) * 256],
                    start=(dc == 0), stop=(dc == DC - 1),
                )
            if c == nj - 1:
                nc.vector.tensor_add(ps[:], ps[:], msk_sb[:, j, :])
            at = attnp.tile([128, 256], BF16, tag="attn")
            s_c = stats.tile([128, 1], F32, tag="mx")
            nc.scalar.activation(
                out=at[:], in_=ps[:],
                func=mybir.ActivationFunctionType.Exp,
                bias=0.0, scale=1.0,
                accum_out=s_c[:],
            )
            return at, s_c

        def emit_post(j, c, at, pax):
            # transpose exp chunk -> aT, accumulate A@X [q, d]
            nj = j + 1
            ps_t = pcom.tile([128, 512], BF16, tag="pc", name="ps_t")[:, 0:256]
            for b in range(2):
                nc.tensor.matmul(
                    ps_t[:, b * 128:(b + 1) * 128],
                    at[:, b * 128:(b + 1) * 128],
                    ident[:],
                    is_transpose=True,
                    start=True, stop=True,
                )
            aT = attnp.tile([128, 2, 128], BF16, tag="attn", name="aT")
            nc.vector.tensor_copy(
                aT[:], ps_t[:].rearrange("p (b q) -> p b q", b=2))
            for bb in range(2):
                kb = c * 2 + bb
                for u in range(2):
                    nc.tensor.matmul(
                        pax[:, u * 512:(u + 1) * 512],
                        aT[:, bb, :],
                        xR_sb[:, kb, u * 512:(u + 1) * 512],
                        start=(kb == 0), stop=(kb == 2 * nj - 1),
                    )

        def emit_tail(j, axs, sums):
            # combine row sums -> 1/denom  [q,1]
            while len(sums) > 1:
                nxt = []
                for i in range(0, len(sums) - 1, 2):
                    s_c = stats.tile([128, 1], F32, tag="mx")
                    nc.vector.tensor_add(s_c[:], sums[i][:], sums[i + 1][:])
                    nxt.append(s_c)
                if len(sums) % 2:
                    nxt.append(sums[-1])
                sums = nxt
            recip = stats.tile([128, 1], F32, tag="mx")
            nc.vector.reciprocal(recip[:], sums[0][:])

            # transpose A@X -> (AX)^T [d, q] and (A X) Wv, pipelined per
            # dc-half so the final tail's copies overlap PE work
            axt = axtsp.tile([128, DC, 128], BF16, tag="axt")
            po = pgpo.tile([128, DC, 128], F32, tag="gp", name="po")
            pof = po[:].rearrange("p m q -> p (m q)")
            for g in range(2):
                ps_t = pcom.tile([128, 512], BF16, tag="pc", name="ps_tx")
                for mm in range(4):
                    nc.tensor.matmul(
                        ps_t[:, mm * 128:(mm + 1) * 128],
                        axs[:, g * 4 + mm, :],
                        ident[:],
                        is_transpose=True,
                        start=True, stop=True,
                    )
                nc.vector.tensor_copy(
                    axt[:, g * 4:(g + 1) * 4, :],
                    ps_t[:].rearrange("p (m q) -> p m q", m=4))
                for dc in range(g * 4, g * 4 + 4):
                    for u in range(2):
                        nc.tensor.matmul(
                            pof[:, u * 512:(u + 1) * 512],
                            axt[:, dc, :],
                            wv_sb[:, dc, u * 512:(u + 1) * 512],
                            start=(dc == 0), stop=(dc == DC - 1),
                        )
            ot = outp.tile([128, D], BF16, tag="o")
            for u in range(2):
                nc.vector.tensor_scalar_mul(
                    ot[:, u * 512:(u + 1) * 512],
                    pof[:, u * 512:(u + 1) * 512], recip[:])
            nc.sync.dma_start(out_d[j * 128:(j + 1) * 128, :], ot[:])

        pending = None
        for step in range(QT_PER_CORE):
            if step + 2 < QT_PER_CORE:
                emit_g(step + 2)
            # staggered chunk pipeline: scores(c+1) + G transposes cover
            # exp(c) latency
            pax = paxp.tile([128, 1024], F32, tag="ax", name="pax")
            sums = []
            prev = None
            for c in range(step + 1):
                at, s_c = emit_scores(step, c)
                sums.append(s_c)
                if prev is not None:
                    emit_post(step, prev[0], prev[1], pax)
                prev = (c, at)
            if pending is not None:
                emit_tail(*pending)          # covers the last chunk's exp
            emit_post(step, prev[0], prev[1], pax)
            axs = axsp.tile([128, DC, 128], BF16, tag="axs")
            paxv = pax[:].rearrange("p (m d) -> p m d", m=DC)
            for g in range(2):
                nc.scalar.copy(
                    axs[:, g * 4:(g + 1) * 4, :], paxv[:, g * 4:(g + 1) * 4, :])
            pending = (step, axs, sums)
        emit_tail(*pending)


def build_nc(reps=1):
    nc = bacc.Bacc("TRN2", target_bir_lowering=False, debug=False,
                   num_swdge_queues=4)

    nc.xT_d = nc.dram_tensor("xT", [D, T], BF16, kind="ExternalInput")
    nc.xR_d = nc.dram_tensor("xR", [T, D], BF16, kind="ExternalInput")
    nc.xqT_d = nc.dram_tensor("xqT", [D, 1024], BF16, kind="ExternalInput")
    nc.w2_d = nc.dram_tensor("w2", [D, D], BF16, kind="ExternalInput")
    nc.wv_d = nc.dram_tensor("wv", [D, D], BF16, kind="ExternalInput")
    nc.mask_d = nc.dram_tensor("mask", [QT_PER_CORE, 128, 256], BF16,
                               kind="ExternalInput")
    nc.out_d = nc.dram_tensor("out", [1024, D], BF16, kind="ExternalOutput")

    with tile.TileContext(nc) as tc:
        for _rep in range(reps):
            _emit_body(nc, tc)

    nc.compile()
    return nc


def make_in_maps(input_vector, w_q, w_k, w_v):
    import ml_dtypes
    bf16 = ml_dtypes.bfloat16

    input_vector = np.asarray(input_vector, dtype=np.float32)
    w2 = np.ascontiguousarray(
        ((np.asarray(w_q, dtype=np.float32) / np.sqrt(np.float32(D)))
         @ np.asarray(w_k, dtype=np.float32).T).astype(bf16))
    wv = np.ascontiguousarray(np.asarray(w_v, dtype=np.float32).astype(bf16))

    in_maps = []
    qrows_per_core = []
    for core in range(8):
        b = core // 2
        par = core % 2
        qt = [2 * j + par for j in range(QT_PER_CORE)]
        xb = input_vector[b]                                   # [T, D]
        xT = np.ascontiguousarray(xb.T.astype(bf16))           # [D, T]
        xR = np.ascontiguousarray(xb.astype(bf16))             # [T, D]
        qrows = np.concatenate(
            [np.arange(t * 128, (t + 1) * 128) for t in qt])
        xqT = np.ascontiguousarray(xb[qrows].T.astype(bf16))   # [D, 1024]
        mask = np.zeros((QT_PER_CORE, 128, 256), np.float32)
        for j, t in enumerate(qt):
            k0 = j * 256                  # last chunk covers keys [k0, k0+256)
            q_abs = t * 128 + np.arange(128)[:, None]
            k_abs = k0 + np.arange(256)[None, :]
            mask[j] = np.where(k_abs <= q_abs, 0.0, np.float32(NEG_INF))
        in_maps.append({
            "xT": xT, "xR": xR, "xqT": xqT,
            "w2": w2, "wv": wv, "mask": mask.astype(bf16),
        })
        qrows_per_core.append((b, qrows))
    return in_maps, qrows_per_core


def assemble_output(results, qrows_per_core):
    out = np.empty((B, T, D), np.float32)
    for core, (b, qrows) in enumerate(qrows_per_core):
        out[b, qrows] = results[core]["out"].astype(np.float32)
    return out


_NC_CACHE = {}


def kernel(input_vector, w_q, w_k, w_v):
    """Full-input entry point: shards across 8 NeuronCores, returns the
    full [4, 2048, 1024] float32 attention output."""
    from concourse.bass_utils import run_bass_kernel_spmd

    if "nc" not in _NC_CACHE:
        _NC_CACHE["nc"] = build_nc()
    nc = _NC_CACHE["nc"]
    in_maps, qrc = make_in_maps(input_vector, w_q, w_k, w_v)
    res = run_bass_kernel_spmd(nc, in_maps, core_ids=list(range(8)))
    return assemble_output(res.results, qrc)
